# revision 17
# baseline (speedup 1.0000x reference)
"""Trainium2 Bass kernel for LocalSpatioTemporalPooling.

Reference computation (per sample n):
  x: (C=256, T=30, H=64, W=44) fp32
  feats[c,t,s] = mean over the (8,44) spatial stripe s of frame t    # 352-elem mean
  scores[t,s] = || feats[:,t,s] ||_2  (clip eps)                     # reduce over C
  top-2 frames per stripe by score; output[s*C + c] = mean of the 2 selected feats

Sharding: pure data parallel -- one sample per NeuronCore (N=8 = n_cores).

Kernel structure per core (x viewed as (C, T*S*352); 352-groups contiguous in HBM):
  - Stream the sample with an f32->fp16 CASTING gpsimd (SWDGE) DMA: the DMA
    engines write half the bytes into SBUF, halving HBM-side DMA occupancy
    (~2.0us/frame) vs the fp32 stream (~4.0us/frame).  The 120us stream is
    the modeled DMA roofline at 2 bytes/element; fp8 casts were rejected
    because the top-2 score margins (min rank2-rank3 gap 0.06%) sit below
    fp8-induced score noise, and a re-rank/re-read phase costs more than the
    fp16 stream saves.  fp16 quantization adds ~0.03% noise to the sumsq
    scores (verified 0/64 selection flips on the fixed input) and ~0.1% to
    the output values; reduction accum stays fp32.
  - Stream pieces (t0, tcn, s0, sn, na): 2-frame chunks, with a 1-frame +
    half-frame taper at the end so the last DVE chain is short.  Each piece's
    352-group sums split between the DVE -- a packed-fp16 halving chain
    (176/88/44/22 tensor_tensor adds hit the DVE 2x performance mode for
    contiguous 2-byte operands) plus one fp32 tensor_reduce over the last 22,
    ~1.25us/frame for 7 stripes -- and the ACT engine, which accumulate-copies
    the `na` top stripes (~0.67us/group) so DVE stays below the 2.0us/frame
    DMA rate and both engines drain right as the last piece lands.  Results
    land in feats (128, 480) fp32 = [c-block 0 | c-block 1], stripe-major
    (s*30 + t).
  - Scores accumulate DURING the stream: after each chunk's reduce, ACT
    squares that slice and PE accumulates ones^T @ sq into the (1, 240) PSUM
    sumsq (per t-range: c-block 0 matmul starts, c-block 1 stops).  Ordering
    by sumsq == ordering by the reference score (monotonic transform).
  - Top-2 per stripe with no DMA round-trip: per-stripe max via strided
    reduce_max on (1, 8, 30), tie mask via stride-0 broadcast tensor_tensor,
    mask out the max, second reduce_max, then w = (ss >= m2) in bf16.
  - PE broadcast (bf16 ones row) -> wb (128, 480); one scalar_tensor_tensor
    (feats * WSCALE * wb), one strided reduce -> (128, 16), one PE transpose
    -> (16, 128), one copy, one DMA to out (viewed (cb s) c).
"""

import dataclasses

import numpy as np
from contextlib import ExitStack

import concourse.bass as bass
import concourse.tile as tile
import concourse.mybir as mybir
from concourse import bacc
from concourse.bass_utils import run_bass_kernel_spmd
from concourse.masks import make_identity

N, C, T, H, W = 8, 256, 30, 64, 44
S = 8                 # stripes
SH = H // S           # 8 rows per stripe
GROUP = SH * W        # 352 elements per (c, t, s) group
CB = C // 128         # 2 channel blocks
FRAME = H * W         # 2816
WSCALE = 0.5 / GROUP  # top-2 mean of stripe means
BIG = 1.0e30

# stream pieces (t0, tcn, s0, sn, na): na top stripes go to the ACT engine
# (accumulate-copy, ~0.67us/group).  Full 2-frame chunks give ACT one stripe;
# the 1-frame taper gives ACT 2-3 stripes so the DVE drains its backlog and
# feats completes right after the last (half-frame) DMA pieces land.
CHUNKS = [
    [(t, 2, 0, 8, 1) for t in range(0, 30, 2)],                          # c-block 0
    [(t, 2, 0, 8, 1) for t in range(0, 24, 2)]
    + [(24, 1, 0, 8, 2), (25, 1, 0, 8, 2), (26, 1, 0, 8, 3),
       (27, 1, 0, 8, 3), (28, 1, 0, 8, 2),
       (29, 1, 0, 4, 0), (29, 1, 4, 8, 0)],                              # c-block 1
]

_F32 = mybir.dt.float32
_BF16 = mybir.dt.bfloat16
_F16 = mybir.dt.float16


def _bcast(ap2d, inner):
    """(1, K) AP -> (1, K, inner) stride-0 broadcast view."""
    [pp, pc], [fs, fc] = ap2d.ap[0], ap2d.ap[1]
    return dataclasses.replace(ap2d, ap=[[pp, pc], [fs, fc], [0, inner]])


def _kernel_body(ctx, tc, nc, x, out, repeat=1):
    const_pool = ctx.enter_context(tc.tile_pool(name="const", bufs=1))
    in_pool = ctx.enter_context(tc.tile_pool(name="inp", bufs=4))
    feat_pool = ctx.enter_context(tc.tile_pool(name="feat", bufs=1))
    small_pool = ctx.enter_context(tc.tile_pool(name="small", bufs=1))
    psum_pool = ctx.enter_context(tc.tile_pool(name="psum", bufs=1, space="PSUM"))

    ones_col = const_pool.tile([128, 1], _F32)
    nc.vector.memset(ones_col[:], 1.0)
    wrow = const_pool.tile([1, 128], _BF16)
    nc.vector.memset(wrow[:], 1.0)
    identity = const_pool.tile([128, 128], _F32)
    make_identity(nc, identity[:])

    # [c-block 0 | c-block 1] side by side; free layout within a block: s*30 + t
    feats = feat_pool.tile([128, CB * T * S], _F32)
    sq = feat_pool.tile([128, CB * T * S], _F32)
    ss_psum = psum_pool.tile([1, T * S], _F32, tag="ss")
    ssv = ss_psum[:].rearrange("p (s t) -> p s t", s=S)

    def fview(cb, t0, tc_, s0=0, sn=S):  # (128, tc_, sn) t-minor slice
        return feats[:, cb * T * S:(cb + 1) * T * S].rearrange(
            "p (s t) -> p t s", s=S)[:, t0:t0 + tc_, s0:s0 + sn]

    # halving-chain scratch (fp16, packed so the DVE 2x mode applies)
    half_pool = ctx.enter_context(tc.tile_pool(name="half", bufs=2))

    # ACT-side accumulate-copy scratch (value discarded, accum kept)
    act_scratch = [feat_pool.tile([128, GROUP], _F32, name=f"actscr{i}")
                   for i in range(2)]
    act_n = [0]

    def act_group_sum(cb, tl, t0, t_local, s_, piece_s0, piece_ns):
        # sum one (c, t, s) 352-group on the Scalar engine via accum_out
        scr = act_scratch[act_n[0] % 2]
        act_n[0] += 1
        g = t_local * piece_ns + (s_ - piece_s0)
        nc.scalar.activation(
            scr[:], tl[:, g * GROUP:(g + 1) * GROUP],
            mybir.ActivationFunctionType.Copy,
            accum_out=feats[:, cb * T * S + s_ * T + (t0 + t_local):
                            cb * T * S + s_ * T + (t0 + t_local) + 1],
        )

    SD = S - 1  # max stripes on the DVE halving chain (sizes the h tiles)

    def group_sums(cb, t0, tcn, s0, sn, na, tl, in4):
        # DVE: fp16 halving adds (2x packed mode) + one fp32 reduce for
        # stripes [s0, sn-na); ACT accumulates the na top stripes so DVE
        # stays below the 2.0us/frame DMA rate and catches up in the taper.
        sdn = sn - na
        nsd = sdn - s0
        ns = sn - s0
        if nsd > 0:
            sv = in4[:, :, :nsd, :]
            h1 = half_pool.tile([128, 2 * SD * (GROUP // 2)], _F16, tag="h1")
            v1 = h1[:, :tcn * nsd * (GROUP // 2)].rearrange(
                "p (t s w) -> p t s w", t=tcn, s=nsd)
            nc.vector.tensor_tensor(v1, sv[:, :, :, :176], sv[:, :, :, 176:],
                                    op=mybir.AluOpType.add)
            h2 = half_pool.tile([128, 2 * SD * (GROUP // 4)], _F16, tag="h2")
            v2 = h2[:, :tcn * nsd * (GROUP // 4)].rearrange(
                "p (t s w) -> p t s w", t=tcn, s=nsd)
            nc.vector.tensor_tensor(v2, v1[:, :, :, :88], v1[:, :, :, 88:],
                                    op=mybir.AluOpType.add)
            h3 = half_pool.tile([128, 2 * SD * (GROUP // 8)], _F16, tag="h3")
            v3 = h3[:, :tcn * nsd * (GROUP // 8)].rearrange(
                "p (t s w) -> p t s w", t=tcn, s=nsd)
            nc.vector.tensor_tensor(v3, v2[:, :, :, :44], v2[:, :, :, 44:],
                                    op=mybir.AluOpType.add)
            h4 = half_pool.tile([128, 2 * SD * (GROUP // 16)], _F16, tag="h4")
            v4 = h4[:, :tcn * nsd * (GROUP // 16)].rearrange(
                "p (t s w) -> p t s w", t=tcn, s=nsd)
            nc.vector.tensor_tensor(v4, v3[:, :, :, :22], v3[:, :, :, 22:],
                                    op=mybir.AluOpType.add)
            nc.vector.tensor_reduce(fview(cb, t0, tcn, s0, nsd), v4,
                                    axis=mybir.AxisListType.X, op=mybir.AluOpType.add)
        for tloc in range(tcn):
            for s_ in range(sdn, sn):
                act_group_sum(cb, tl, t0, tloc, s_, s0, ns)

    def sview(t_, cb, t0, tc_):  # (128, 8, tc_) s-major slice
        return t_[:, cb * T * S:(cb + 1) * T * S].rearrange(
            "p (s t) -> p s t", s=S)[:, :, t0:t0 + tc_]

    for _rep in range(repeat):
        # ---- streamed reduction + in-stream score accumulation ----
        for cb in range(CB):
            for (t0, tcn, s0, sn, na) in CHUNKS[cb]:
                ns = sn - s0
                tl = in_pool.tile([128, 2 * S * GROUP], _F16, name="tl", tag="tl")
                nc.gpsimd.dma_start(
                    tl[:, :tcn * ns * GROUP],
                    x[cb * 128:(cb + 1) * 128,
                      t0 * FRAME + s0 * GROUP:(t0 + tcn - 1) * FRAME + sn * GROUP],
                )
                in4 = tl[:, :tcn * ns * GROUP].rearrange(
                    "p (t s w) -> p t s w", t=tcn, s=ns)
                group_sums(cb, t0, tcn, s0, sn, na, tl, in4)
                if sn < S:
                    continue  # sq/matmul fire on the t-range's last piece
                nc.scalar.activation(
                    sview(sq, cb, t0, tcn), sview(feats, cb, t0, tcn),
                    mybir.ActivationFunctionType.Square,
                )
                if cb == 1:
                    # both c-blocks' squares for this t-range are now available
                    nc.tensor.matmul(
                        ssv[:, :, t0:t0 + tcn], lhsT=ones_col[:],
                        rhs=sview(sq, 0, t0, tcn), start=True, stop=False,
                    )
                    nc.tensor.matmul(
                        ssv[:, :, t0:t0 + tcn], lhsT=ones_col[:],
                        rhs=sview(sq, 1, t0, tcn), start=False, stop=True,
                    )

        # ---- per-stripe top-2 -> weight row (all on partition 0, no DMA) ----
        ss_sb = small_pool.tile([1, T * S], _F32)
        nc.vector.tensor_copy(ss_sb[:], ss_psum[:])
        ssv_sb = ss_sb[:].rearrange("p (s t) -> p s t", s=S)
        m1 = small_pool.tile([1, S], _F32)
        nc.vector.tensor_reduce(m1[:], ssv_sb, axis=mybir.AxisListType.X,
                                op=mybir.AluOpType.max)
        eq1 = small_pool.tile([1, T * S], _F32)
        eq1v = eq1[:].rearrange("p (s t) -> p s t", s=S)
        nc.vector.tensor_tensor(eq1v, ssv_sb, _bcast(m1[:], T), op=mybir.AluOpType.is_ge)
        masked = small_pool.tile([1, T * S], _F32)
        maskedv = masked[:].rearrange("p (s t) -> p s t", s=S)
        nc.vector.scalar_tensor_tensor(
            maskedv, eq1v, -BIG, ssv_sb,
            op0=mybir.AluOpType.mult, op1=mybir.AluOpType.add,
        )
        m2 = small_pool.tile([1, S], _F32)
        nc.vector.tensor_reduce(m2[:], maskedv, axis=mybir.AxisListType.X,
                                op=mybir.AluOpType.max)
        w = small_pool.tile([1, T * S], _BF16)
        wv = w[:].rearrange("p (s t) -> p s t", s=S)
        nc.vector.tensor_tensor(wv, ssv_sb, _bcast(m2[:], T), op=mybir.AluOpType.is_ge)

        # ---- weighted frame mean + output assembly (single fused pass) ----
        wb_psum = psum_pool.tile([128, CB * T * S], _F32, tag="wb")
        for cb in range(CB):
            nc.tensor.matmul(wb_psum[:, cb * T * S:(cb + 1) * T * S],
                             lhsT=wrow[:], rhs=w[:], start=True, stop=True)
        prod = small_pool.tile([128, CB * T * S], _F32)
        nc.vector.scalar_tensor_tensor(
            prod[:], feats[:], WSCALE, wb_psum[:],
            op0=mybir.AluOpType.mult, op1=mybir.AluOpType.mult,
        )
        oblk = small_pool.tile([128, CB * S], _F32)
        nc.vector.tensor_reduce(
            oblk[:], prod[:].rearrange("p (b s t) -> p b s t", b=CB, s=S),
            axis=mybir.AxisListType.X, op=mybir.AluOpType.add,
        )
        tr_psum = psum_pool.tile([CB * S, 128], _F32, tag="tr")
        nc.tensor.transpose(tr_psum[:], oblk[:], identity[:])
        outt = small_pool.tile([CB * S, 128], _F32)
        nc.vector.tensor_copy(outt[:], tr_psum[:])
        nc.sync.dma_start(out.rearrange("s (b c) -> b s c", b=CB), outt[:])


_NC_CACHE = {}


def _get_nc(repeat=1):
    if repeat not in _NC_CACHE:
        nc = bacc.Bacc("TRN2", target_bir_lowering=False, debug=False)
        x = nc.dram_tensor("x", [C, T * FRAME], _F32, kind="ExternalInput").ap()
        out = nc.dram_tensor("out", [S, C], _F32, kind="ExternalOutput").ap()
        with tile.TileContext(nc) as tc:
            with ExitStack() as ctx:
                _kernel_body(ctx, tc, nc, x, out, repeat=repeat)
        nc.compile()
        _NC_CACHE[repeat] = nc
    return _NC_CACHE[repeat]


def kernel(x):
    x = np.asarray(x, dtype=np.float32)
    assert x.shape == (N, C, T, H, W), x.shape
    nc = _get_nc()
    in_maps = [{"x": np.ascontiguousarray(x[i]).reshape(C, T * FRAME)} for i in range(N)]
    res = run_bass_kernel_spmd(nc, in_maps, list(range(N)))
    return np.stack([res.results[i]["out"].reshape(S * C) for i in range(N)])
